# revision 11
# baseline (speedup 1.0000x reference)
"""Expert-parallel MoE (DBRX-style SwiGLU FFN) on 8 TRN2 NeuronCores.

Strategy: one expert per core. Routing (gather tokens per expert, combine
weights) happens on the host; each core runs the SwiGLU FFN for its expert
over its gathered tokens in "transposed activation" form:

    up^T   = w_up^T  @ x^T    (K = H, accumulate over 8 K-subtiles)
    gate^T = w_gate^T @ x^T
    h^T    = silu(up^T) * gate^T          (bf16)
    y^T    = w_down^T @ h^T   (K = F, accumulate over 16 K-subtiles)

All matmuls are bf16 with f32 PSUM accumulation. The host applies the
per-(token, expert) combine weight during the scatter-add.

Schedule notes (from NTFF profile analysis):
  - ~6.6 us of framework preamble runs before any user instruction; DMA
    kicks can't beat it. First data packets land ~1.5 us after the kick.
  - HAM un-throttles the PE clock ~5.3 us after sustained PE activity
    begins; junk matmuls start that window and cover the initial DMA.
  - The PE stream (LDWEIGHTS hidden under matmuls) runs at ~213 ns per
    512-wide matmul; keep it saturated and trim everything around it.
"""

import numpy as np
import ml_dtypes

import concourse.bacc as bacc
import concourse.mybir as mybir
import concourse.tile as tile
from concourse import bass_utils

HIDDEN = 1024
FFN = 2048
N_EXPERTS = 8
P = 128
KO_H = HIDDEN // P   # 8   K-subtiles for up/gate
KO_F = FFN // P      # 16  K-subtiles for down
FC_N = FFN // P      # 16  F-chunks (output partition tiles of stage A)
HC_N = HIDDEN // P   # 8   H-chunks (output partition tiles of stage B)

WARM_N = 52          # junk matmuls (N=128) covering preamble->first-data

BF16 = ml_dtypes.bfloat16

_compiled = {}  # cap -> compiled Bacc module


def _build(cap: int):
    f32 = mybir.dt.float32
    bf16 = mybir.dt.bfloat16
    tchunks = [(t0, min(512, cap - t0)) for t0 in range(0, cap, 512)]

    nc = bacc.Bacc("TRN2", debug=False, enable_asserts=False,
                   num_devices=N_EXPERTS)
    xT_d = nc.dram_tensor("xT", [P, KO_H, cap], bf16, kind="ExternalInput")
    wu_d = nc.dram_tensor("wu", [FC_N, P, KO_H, P], bf16, kind="ExternalInput")
    wg_d = nc.dram_tensor("wg", [FC_N, P, KO_H, P], bf16, kind="ExternalInput")
    wd_d = nc.dram_tensor("wd", [HC_N, P, KO_F, P], bf16, kind="ExternalInput")
    yT_d = nc.dram_tensor("yT", [HC_N, P, cap], bf16, kind="ExternalOutput")

    with tile.TileContext(nc) as tc:
        with (
            tc.tile_pool(name="persist", bufs=1) as persist,
            tc.tile_pool(name="wpool", bufs=3) as wpool,
            tc.tile_pool(name="spool", bufs=4) as spool,
            tc.tile_pool(name="psum", bufs=2, space="PSUM") as psum,
        ):
            # PE clock warm-up: junk matmuls keep the PE busy from the end
            # of the engine preamble until the first real inputs land, so
            # the HAM un-throttle window (~5.3 us of sustained activity)
            # elapses while DMA streams in. N=128 keeps each one short so
            # the junk ends right as data becomes available.
            warm = persist.tile([P, P], bf16, tag="warm")
            nc.vector.memset(warm[:], 0)
            pwarm = psum.tile([P, 512], f32, tag="pwarm", name="pwarm")
            for _ in range(WARM_N):
                nc.tensor.matmul(pwarm[:, :P], warm, warm, start=True,
                                 stop=True)

            xT = persist.tile([P, KO_H, cap], bf16, tag="xT")
            ht = persist.tile([P, KO_F, cap], bf16, tag="ht")

            # Stage A: h^T[fc] = silu(up^T) * gate^T, per 128-wide F-chunk
            for fc in range(FC_N):
                wu_t = wpool.tile([P, KO_H, P], bf16, tag="wu")
                wg_t = wpool.tile([P, KO_H, P], bf16, tag="wg")
                if fc == 0:
                    # x^T quarter 0 first (largest piece of the first
                    # matmul's dependency set), weights next, then the
                    # remaining quarters in consumption order
                    nc.sync.dma_start(xT[:, 0:2], xT_d.ap()[:, 0:2])
                    nc.sync.dma_start(wu_t[:], wu_d.ap()[fc])
                    nc.sync.dma_start(wg_t[:], wg_d.ap()[fc])
                    for q in range(1, 4):
                        nc.sync.dma_start(xT[:, 2 * q:2 * q + 2],
                                          xT_d.ap()[:, 2 * q:2 * q + 2])
                else:
                    nc.sync.dma_start(wu_t[:], wu_d.ap()[fc])
                    nc.sync.dma_start(wg_t[:], wg_d.ap()[fc])
                for t0, nt in tchunks:
                    pu = psum.tile([P, 512], f32, tag="pu", name="pu")[:, :nt]
                    pg = psum.tile([P, 512], f32, tag="pg", name="pg")[:, :nt]
                    # interleave the two accumulation chains so each x^T
                    # quarter is fully consumed as soon as it lands —
                    # pulls the end of fc0 forward during the DMA ramp
                    for ko in range(KO_H):
                        nc.tensor.matmul(pu, wu_t[:, ko], xT[:, ko, t0:t0 + nt],
                                         start=(ko == 0), stop=(ko == KO_H - 1))
                        nc.tensor.matmul(pg, wg_t[:, ko], xT[:, ko, t0:t0 + nt],
                                         start=(ko == 0), stop=(ko == KO_H - 1))
                    su = spool.tile([P, 512], f32, tag="silu", name="su")[:, :nt]
                    nc.scalar.activation(su, pu,
                                         mybir.ActivationFunctionType.Sigmoid)
                    nc.vector.tensor_mul(su, su, pu)
                    nc.vector.tensor_mul(ht[:, fc, t0:t0 + nt], su, pg)

            # Stage B: y^T[hc] = w_down^T @ h^T, per 128-wide H-chunk
            for hc in range(HC_N):
                wd_t = wpool.tile([P, KO_F, P], bf16, tag="wd")
                nc.sync.dma_start(wd_t[:], wd_d.ap()[hc])
                # split the last chunk's tokens so the final PSUM-drain +
                # DMA-out chain after the very last matmul is shorter
                chunks = tchunks
                if hc == HC_N - 1 and tchunks[-1][1] > 256:
                    t0l, ntl = tchunks[-1]
                    chunks = tchunks[:-1] + [(t0l, 256), (t0l + 256, 128),
                                             (t0l + 384, ntl - 384)]
                for t0, nt in chunks:
                    py = psum.tile([P, 512], f32, tag="py", name="py")[:, :nt]
                    for ko in range(KO_F):
                        nc.tensor.matmul(py, wd_t[:, ko], ht[:, ko, t0:t0 + nt],
                                         start=(ko == 0), stop=(ko == KO_F - 1))
                    yo = spool.tile([P, 512], bf16, tag="yo", name="yo")[:, :nt]
                    nc.vector.tensor_copy(yo, py)
                    nc.sync.dma_start(yT_d.ap()[hc][:, t0:t0 + nt], yo)

    nc.compile()
    return nc


def kernel(x, weights, top_weights, top_experts, w_up, w_gate, w_down):
    x = np.asarray(x, dtype=np.float32)
    tw = np.asarray(top_weights, dtype=np.float32)
    te = np.asarray(top_experts).astype(np.int64)
    w_up = np.asarray(w_up, dtype=np.float32)
    w_gate = np.asarray(w_gate, dtype=np.float32)
    w_down = np.asarray(w_down, dtype=np.float32)

    B, S, H = x.shape
    T = B * S
    xf = x.reshape(T, H)

    # --- host routing ---
    idxs, combine = [], []
    for e in range(N_EXPERTS):
        sel = te == e                       # [T, K]
        mask = sel.any(axis=1)
        idx = np.nonzero(mask)[0]
        w_tok = (tw * sel).sum(axis=1)      # [T]
        idxs.append(idx)
        combine.append(w_tok[idx].astype(np.float32))
    max_n = max(len(i) for i in idxs)
    cap = max(max_n, P)  # exact token capacity; moving dim need not be 128k

    # --- per-core inputs ---
    in_maps = []
    for e in range(N_EXPERTS):
        idx = idxs[e]
        xg = np.zeros((cap, H), np.float32)
        xg[: len(idx)] = xf[idx]
        # xT[p, ko, t] = xg[t, ko*128+p]
        xT = np.ascontiguousarray(
            xg.T.reshape(KO_H, P, cap).transpose(1, 0, 2)).astype(BF16)
        wu = np.ascontiguousarray(
            w_up[e].reshape(KO_H, P, FC_N, P).transpose(2, 1, 0, 3)).astype(BF16)
        wg = np.ascontiguousarray(
            w_gate[e].reshape(KO_H, P, FC_N, P).transpose(2, 1, 0, 3)).astype(BF16)
        wd = np.ascontiguousarray(
            w_down[e].reshape(KO_F, P, HC_N, P).transpose(2, 1, 0, 3)).astype(BF16)
        in_maps.append({"xT": xT, "wu": wu, "wg": wg, "wd": wd})

    # --- compile (cached) + run ---
    if cap not in _compiled:
        _compiled[cap] = _build(cap)
    nc = _compiled[cap]
    res = bass_utils.run_bass_kernel_spmd(
        nc, in_maps, core_ids=list(range(N_EXPERTS)))

    # --- combine on host ---
    out = np.zeros((T, H), np.float32)
    for e in range(N_EXPERTS):
        idx = idxs[e]
        yT = res.results[e]["yT"].astype(np.float32).reshape(H, cap)
        out[idx] += yT[:, : len(idx)].T * combine[e][:, None]
    return out.reshape(B, S, H)


# revision 12
# speedup vs baseline: 1.0078x; 1.0078x over previous
"""Expert-parallel MoE (DBRX-style SwiGLU FFN) on 8 TRN2 NeuronCores.

Strategy: one expert per core. Routing (gather tokens per expert, combine
weights) happens on the host; each core runs the SwiGLU FFN for its expert
over its gathered tokens in "transposed activation" form:

    up^T   = w_up^T  @ x^T    (K = H, accumulate over 8 K-subtiles)
    gate^T = w_gate^T @ x^T
    h^T    = silu(up^T) * gate^T          (bf16)
    y^T    = w_down^T @ h^T   (K = F, accumulate over 16 K-subtiles)

All matmuls are bf16 with f32 PSUM accumulation. The host applies the
per-(token, expert) combine weight during the scatter-add.

Schedule notes (from NTFF profile analysis):
  - ~6.6 us of framework preamble runs before any user instruction; DMA
    kicks can't beat it. First data packets land ~1.5 us after the kick.
  - HAM un-throttles the PE clock ~5.3 us after sustained PE activity
    begins; junk matmuls start that window and cover the initial DMA.
  - The PE stream (LDWEIGHTS hidden under matmuls) runs at ~213 ns per
    512-wide matmul; keep it saturated and trim everything around it.
"""

import numpy as np
import ml_dtypes

import concourse.bacc as bacc
import concourse.mybir as mybir
import concourse.tile as tile
from concourse import bass_utils

HIDDEN = 1024
FFN = 2048
N_EXPERTS = 8
P = 128
KO_H = HIDDEN // P   # 8   K-subtiles for up/gate
KO_F = FFN // P      # 16  K-subtiles for down
FC_N = FFN // P      # 16  F-chunks (output partition tiles of stage A)
HC_N = HIDDEN // P   # 8   H-chunks (output partition tiles of stage B)

WARM_N = 60          # junk matmuls (N=128) covering preamble->first-data

BF16 = ml_dtypes.bfloat16

_compiled = {}  # cap -> compiled Bacc module


def _build(cap: int):
    f32 = mybir.dt.float32
    bf16 = mybir.dt.bfloat16
    tchunks = [(t0, min(512, cap - t0)) for t0 in range(0, cap, 512)]

    nc = bacc.Bacc("TRN2", debug=False, enable_asserts=False,
                   num_devices=N_EXPERTS)
    xT_d = nc.dram_tensor("xT", [P, KO_H, cap], bf16, kind="ExternalInput")
    wu_d = nc.dram_tensor("wu", [FC_N, P, KO_H, P], bf16, kind="ExternalInput")
    wg_d = nc.dram_tensor("wg", [FC_N, P, KO_H, P], bf16, kind="ExternalInput")
    wd_d = nc.dram_tensor("wd", [HC_N, P, KO_F, P], bf16, kind="ExternalInput")
    yT_d = nc.dram_tensor("yT", [HC_N, P, cap], bf16, kind="ExternalOutput")

    with tile.TileContext(nc) as tc:
        with (
            tc.tile_pool(name="persist", bufs=1) as persist,
            tc.tile_pool(name="wpool", bufs=3) as wpool,
            tc.tile_pool(name="spool", bufs=4) as spool,
            tc.tile_pool(name="psum", bufs=2, space="PSUM") as psum,
        ):
            # PE clock warm-up: junk matmuls keep the PE busy from the end
            # of the engine preamble until the first real inputs land, so
            # the HAM un-throttle window (~5.3 us of sustained activity)
            # elapses while DMA streams in. N=128 keeps each one short so
            # the junk ends right as data becomes available.
            warm = persist.tile([P, P], bf16, tag="warm")
            nc.vector.memset(warm[:], 0)
            pwarm = psum.tile([P, 512], f32, tag="pwarm", name="pwarm")
            for _ in range(WARM_N):
                nc.tensor.matmul(pwarm[:, :P], warm, warm, start=True,
                                 stop=True)

            xT = persist.tile([P, KO_H, cap], bf16, tag="xT")
            ht = persist.tile([P, KO_F, cap], bf16, tag="ht")

            # Stage A: h^T[fc] = silu(up^T) * gate^T, per 128-wide F-chunk
            for fc in range(FC_N):
                wu_t = wpool.tile([P, KO_H, P], bf16, tag="wu")
                wg_t = wpool.tile([P, KO_H, P], bf16, tag="wg")
                if fc == 0:
                    # x^T quarter 0 first (largest piece of the first
                    # matmul's dependency set), weights next, then the
                    # remaining quarters in consumption order
                    nc.sync.dma_start(xT[:, 0:2], xT_d.ap()[:, 0:2])
                    nc.sync.dma_start(wu_t[:], wu_d.ap()[fc])
                    nc.sync.dma_start(wg_t[:], wg_d.ap()[fc])
                    for q in range(1, 4):
                        nc.sync.dma_start(xT[:, 2 * q:2 * q + 2],
                                          xT_d.ap()[:, 2 * q:2 * q + 2])
                else:
                    nc.sync.dma_start(wu_t[:], wu_d.ap()[fc])
                    nc.sync.dma_start(wg_t[:], wg_d.ap()[fc])
                for t0, nt in tchunks:
                    pu = psum.tile([P, 512], f32, tag="pu", name="pu")[:, :nt]
                    pg = psum.tile([P, 512], f32, tag="pg", name="pg")[:, :nt]
                    # interleave the two accumulation chains so each x^T
                    # quarter is fully consumed as soon as it lands —
                    # pulls the end of fc0 forward during the DMA ramp
                    for ko in range(KO_H):
                        nc.tensor.matmul(pu, wu_t[:, ko], xT[:, ko, t0:t0 + nt],
                                         start=(ko == 0), stop=(ko == KO_H - 1))
                        nc.tensor.matmul(pg, wg_t[:, ko], xT[:, ko, t0:t0 + nt],
                                         start=(ko == 0), stop=(ko == KO_H - 1))
                    su = spool.tile([P, 512], f32, tag="silu", name="su")[:, :nt]
                    nc.scalar.activation(su, pu,
                                         mybir.ActivationFunctionType.Sigmoid)
                    nc.vector.tensor_mul(su, su, pu)
                    nc.vector.tensor_mul(ht[:, fc, t0:t0 + nt], su, pg)

            # Stage B: y^T[hc] = w_down^T @ h^T, per 128-wide H-chunk
            for hc in range(HC_N):
                wd_t = wpool.tile([P, KO_F, P], bf16, tag="wd")
                nc.sync.dma_start(wd_t[:], wd_d.ap()[hc])
                # split the last chunk's tokens so the final PSUM-drain +
                # DMA-out chain after the very last matmul is shorter
                chunks = tchunks
                if hc == HC_N - 1 and tchunks[-1][1] > 256:
                    t0l, ntl = tchunks[-1]
                    chunks = tchunks[:-1] + [(t0l, 256), (t0l + 256, 128),
                                             (t0l + 384, ntl - 384)]
                for t0, nt in chunks:
                    py = psum.tile([P, 512], f32, tag="py", name="py")[:, :nt]
                    for ko in range(KO_F):
                        nc.tensor.matmul(py, wd_t[:, ko], ht[:, ko, t0:t0 + nt],
                                         start=(ko == 0), stop=(ko == KO_F - 1))
                    yo = spool.tile([P, 512], bf16, tag="yo", name="yo")[:, :nt]
                    nc.vector.tensor_copy(yo, py)
                    nc.sync.dma_start(yT_d.ap()[hc][:, t0:t0 + nt], yo)

    nc.compile()
    return nc


def kernel(x, weights, top_weights, top_experts, w_up, w_gate, w_down):
    x = np.asarray(x, dtype=np.float32)
    tw = np.asarray(top_weights, dtype=np.float32)
    te = np.asarray(top_experts).astype(np.int64)
    w_up = np.asarray(w_up, dtype=np.float32)
    w_gate = np.asarray(w_gate, dtype=np.float32)
    w_down = np.asarray(w_down, dtype=np.float32)

    B, S, H = x.shape
    T = B * S
    xf = x.reshape(T, H)

    # --- host routing ---
    idxs, combine = [], []
    for e in range(N_EXPERTS):
        sel = te == e                       # [T, K]
        mask = sel.any(axis=1)
        idx = np.nonzero(mask)[0]
        w_tok = (tw * sel).sum(axis=1)      # [T]
        idxs.append(idx)
        combine.append(w_tok[idx].astype(np.float32))
    max_n = max(len(i) for i in idxs)
    cap = max(max_n, P)  # exact token capacity; moving dim need not be 128k

    # --- per-core inputs ---
    in_maps = []
    for e in range(N_EXPERTS):
        idx = idxs[e]
        xg = np.zeros((cap, H), np.float32)
        xg[: len(idx)] = xf[idx]
        # xT[p, ko, t] = xg[t, ko*128+p]
        xT = np.ascontiguousarray(
            xg.T.reshape(KO_H, P, cap).transpose(1, 0, 2)).astype(BF16)
        wu = np.ascontiguousarray(
            w_up[e].reshape(KO_H, P, FC_N, P).transpose(2, 1, 0, 3)).astype(BF16)
        wg = np.ascontiguousarray(
            w_gate[e].reshape(KO_H, P, FC_N, P).transpose(2, 1, 0, 3)).astype(BF16)
        wd = np.ascontiguousarray(
            w_down[e].reshape(KO_F, P, HC_N, P).transpose(2, 1, 0, 3)).astype(BF16)
        in_maps.append({"xT": xT, "wu": wu, "wg": wg, "wd": wd})

    # --- compile (cached) + run ---
    if cap not in _compiled:
        _compiled[cap] = _build(cap)
    nc = _compiled[cap]
    res = bass_utils.run_bass_kernel_spmd(
        nc, in_maps, core_ids=list(range(N_EXPERTS)))

    # --- combine on host ---
    out = np.zeros((T, H), np.float32)
    for e in range(N_EXPERTS):
        idx = idxs[e]
        yT = res.results[e]["yT"].astype(np.float32).reshape(H, cap)
        out[idx] += yT[:, : len(idx)].T * combine[e][:, None]
    return out.reshape(B, S, H)


# revision 13
# speedup vs baseline: 1.0157x; 1.0078x over previous
"""Expert-parallel MoE (DBRX-style SwiGLU FFN) on 8 TRN2 NeuronCores.

Strategy: one expert per core. Routing (gather tokens per expert, combine
weights) happens on the host; each core runs the SwiGLU FFN for its expert
over its gathered tokens in "transposed activation" form:

    up^T   = w_up^T  @ x^T    (K = H, accumulate over 8 K-subtiles)
    gate^T = w_gate^T @ x^T
    h^T    = silu(up^T) * gate^T          (bf16)
    y^T    = w_down^T @ h^T   (K = F, accumulate over 16 K-subtiles)

All matmuls are bf16 with f32 PSUM accumulation. The host applies the
per-(token, expert) combine weight during the scatter-add.

Schedule notes (from NTFF profile analysis):
  - ~6.6 us of framework preamble runs before any user instruction; DMA
    kicks can't beat it. First data packets land ~1.5 us after the kick.
  - HAM un-throttles the PE clock ~5.3 us after sustained PE activity
    begins; junk matmuls start that window and cover the initial DMA.
  - The PE stream (LDWEIGHTS hidden under matmuls) runs at ~213 ns per
    512-wide matmul; keep it saturated and trim everything around it.
"""

import numpy as np
import ml_dtypes

import concourse.bacc as bacc
import concourse.mybir as mybir
import concourse.tile as tile
from concourse import bass_utils

HIDDEN = 1024
FFN = 2048
N_EXPERTS = 8
P = 128
KO_H = HIDDEN // P   # 8   K-subtiles for up/gate
KO_F = FFN // P      # 16  K-subtiles for down
FC_N = FFN // P      # 16  F-chunks (output partition tiles of stage A)
HC_N = HIDDEN // P   # 8   H-chunks (output partition tiles of stage B)

WARM_N = 60          # junk matmuls (N=128) covering preamble->first-data

BF16 = ml_dtypes.bfloat16

_compiled = {}  # cap -> compiled Bacc module


def _build(cap: int):
    f32 = mybir.dt.float32
    bf16 = mybir.dt.bfloat16
    tchunks = [(t0, min(512, cap - t0)) for t0 in range(0, cap, 512)]

    nc = bacc.Bacc("TRN2", debug=False, enable_asserts=False,
                   num_devices=N_EXPERTS)
    xT_d = nc.dram_tensor("xT", [P, KO_H, cap], bf16, kind="ExternalInput")
    wu_d = nc.dram_tensor("wu", [FC_N, P, KO_H, P], bf16, kind="ExternalInput")
    wg_d = nc.dram_tensor("wg", [FC_N, P, KO_H, P], bf16, kind="ExternalInput")
    wd_d = nc.dram_tensor("wd", [HC_N, P, KO_F, P], bf16, kind="ExternalInput")
    yT_d = nc.dram_tensor("yT", [HC_N, P, cap], bf16, kind="ExternalOutput")

    with tile.TileContext(nc) as tc:
        with (
            tc.tile_pool(name="persist", bufs=1) as persist,
            tc.tile_pool(name="wpool", bufs=3) as wpool,
            tc.tile_pool(name="spool", bufs=4) as spool,
            tc.tile_pool(name="psum", bufs=2, space="PSUM") as psum,
        ):
            # PE clock warm-up: junk matmuls keep the PE busy from the end
            # of the engine preamble until the first real inputs land, so
            # the HAM un-throttle window (~5.3 us of sustained activity)
            # elapses while DMA streams in. N=128 keeps each one short so
            # the junk ends right as data becomes available.
            warm = persist.tile([P, P], bf16, tag="warm")
            nc.vector.memset(warm[:], 0)
            pwarm = psum.tile([P, 512], f32, tag="pwarm", name="pwarm")
            for _ in range(WARM_N):
                nc.tensor.matmul(pwarm[:, :P], warm, warm, start=True,
                                 stop=True)

            xT = persist.tile([P, KO_H, cap], bf16, tag="xT")
            ht = persist.tile([P, KO_F, cap], bf16, tag="ht")

            # Stage A: h^T[fc] = silu(up^T) * gate^T, per 128-wide F-chunk
            for fc in range(FC_N):
                wu_t = wpool.tile([P, KO_H, P], bf16, tag="wu")
                wg_t = wpool.tile([P, KO_H, P], bf16, tag="wg")
                if fc == 0:
                    # x^T quarter 0 first (largest piece of the first
                    # matmul's dependency set), weights next, then the
                    # remaining quarters in consumption order
                    nc.sync.dma_start(xT[:, 0:2], xT_d.ap()[:, 0:2])
                    nc.sync.dma_start(wu_t[:], wu_d.ap()[fc])
                    nc.sync.dma_start(wg_t[:], wg_d.ap()[fc])
                    for q in range(1, 4):
                        nc.sync.dma_start(xT[:, 2 * q:2 * q + 2],
                                          xT_d.ap()[:, 2 * q:2 * q + 2])
                else:
                    nc.sync.dma_start(wu_t[:], wu_d.ap()[fc])
                    nc.sync.dma_start(wg_t[:], wg_d.ap()[fc])
                for t0, nt in tchunks:
                    pu = psum.tile([P, 512], f32, tag="pu", name="pu")[:, :nt]
                    pg = psum.tile([P, 512], f32, tag="pg", name="pg")[:, :nt]
                    # interleave the two accumulation chains, pu leading
                    # pg by one ko: each x^T quarter is consumed as soon
                    # as it lands, and the first two matmuls need only
                    # x^T[0:2] + w_up (w_gate is the 3rd DMA kick)
                    seq = []
                    for ko in range(KO_H):
                        seq.append((pu, wu_t, ko))
                        if ko >= 1:
                            seq.append((pg, wg_t, ko - 1))
                    seq.append((pg, wg_t, KO_H - 1))
                    for dst, w_t, ko in seq:
                        nc.tensor.matmul(dst, w_t[:, ko], xT[:, ko, t0:t0 + nt],
                                         start=(ko == 0), stop=(ko == KO_H - 1))
                    su = spool.tile([P, 512], f32, tag="silu", name="su")[:, :nt]
                    nc.scalar.activation(su, pu,
                                         mybir.ActivationFunctionType.Sigmoid)
                    nc.vector.tensor_mul(su, su, pu)
                    nc.vector.tensor_mul(ht[:, fc, t0:t0 + nt], su, pg)

            # Stage B: y^T[hc] = w_down^T @ h^T, per 128-wide H-chunk
            for hc in range(HC_N):
                wd_t = wpool.tile([P, KO_F, P], bf16, tag="wd")
                nc.sync.dma_start(wd_t[:], wd_d.ap()[hc])
                # split the last chunk's tokens so the final PSUM-drain +
                # DMA-out chain after the very last matmul is shorter
                chunks = tchunks
                if hc == HC_N - 1 and tchunks[-1][1] > 256:
                    t0l, ntl = tchunks[-1]
                    chunks = tchunks[:-1] + [(t0l, 256), (t0l + 256, 128),
                                             (t0l + 384, ntl - 384)]
                for t0, nt in chunks:
                    py = psum.tile([P, 512], f32, tag="py", name="py")[:, :nt]
                    for ko in range(KO_F):
                        nc.tensor.matmul(py, wd_t[:, ko], ht[:, ko, t0:t0 + nt],
                                         start=(ko == 0), stop=(ko == KO_F - 1))
                    yo = spool.tile([P, 512], bf16, tag="yo", name="yo")[:, :nt]
                    nc.vector.tensor_copy(yo, py)
                    nc.sync.dma_start(yT_d.ap()[hc][:, t0:t0 + nt], yo)

    nc.compile()
    return nc


def kernel(x, weights, top_weights, top_experts, w_up, w_gate, w_down):
    x = np.asarray(x, dtype=np.float32)
    tw = np.asarray(top_weights, dtype=np.float32)
    te = np.asarray(top_experts).astype(np.int64)
    w_up = np.asarray(w_up, dtype=np.float32)
    w_gate = np.asarray(w_gate, dtype=np.float32)
    w_down = np.asarray(w_down, dtype=np.float32)

    B, S, H = x.shape
    T = B * S
    xf = x.reshape(T, H)

    # --- host routing ---
    idxs, combine = [], []
    for e in range(N_EXPERTS):
        sel = te == e                       # [T, K]
        mask = sel.any(axis=1)
        idx = np.nonzero(mask)[0]
        w_tok = (tw * sel).sum(axis=1)      # [T]
        idxs.append(idx)
        combine.append(w_tok[idx].astype(np.float32))
    max_n = max(len(i) for i in idxs)
    cap = max(max_n, P)  # exact token capacity; moving dim need not be 128k

    # --- per-core inputs ---
    in_maps = []
    for e in range(N_EXPERTS):
        idx = idxs[e]
        xg = np.zeros((cap, H), np.float32)
        xg[: len(idx)] = xf[idx]
        # xT[p, ko, t] = xg[t, ko*128+p]
        xT = np.ascontiguousarray(
            xg.T.reshape(KO_H, P, cap).transpose(1, 0, 2)).astype(BF16)
        wu = np.ascontiguousarray(
            w_up[e].reshape(KO_H, P, FC_N, P).transpose(2, 1, 0, 3)).astype(BF16)
        wg = np.ascontiguousarray(
            w_gate[e].reshape(KO_H, P, FC_N, P).transpose(2, 1, 0, 3)).astype(BF16)
        wd = np.ascontiguousarray(
            w_down[e].reshape(KO_F, P, HC_N, P).transpose(2, 1, 0, 3)).astype(BF16)
        in_maps.append({"xT": xT, "wu": wu, "wg": wg, "wd": wd})

    # --- compile (cached) + run ---
    if cap not in _compiled:
        _compiled[cap] = _build(cap)
    nc = _compiled[cap]
    res = bass_utils.run_bass_kernel_spmd(
        nc, in_maps, core_ids=list(range(N_EXPERTS)))

    # --- combine on host ---
    out = np.zeros((T, H), np.float32)
    for e in range(N_EXPERTS):
        idx = idxs[e]
        yT = res.results[e]["yT"].astype(np.float32).reshape(H, cap)
        out[idx] += yT[:, : len(idx)].T * combine[e][:, None]
    return out.reshape(B, S, H)
